# revision 1
# baseline (speedup 1.0000x reference)
"""CenterLoss (segment_reduce) Trainium2 kernel.

Math (faithful to the reference):
  preds = argmax_c logits[n, c, h, w]          (softmax is monotone -> skip it)
  s1[p] = sum_c x, s2[p] = sum_c x^2 per pixel p=(n,h,w)
  per (n, cls): cnt = #pixels with preds==cls, S1 = sum s1, S2 = sum s2
  K = max(cnt,1)*C; sq_dev = max(S2 - S1^2/K, 0)
  loss = sum_cls mean_n( cnt>0 ? sqrt(sq_dev) : 0 )

Device strategy (8 cores, data-parallel over 16 units = (n, H-slab of 128)):
  Each core takes 2 units of shape (C=19, 128, 1024) fp32.  SBUF layout puts
  H on partitions and (C, W) on the free dim, so per-pixel class reductions
  are free-dim ops at full 128-partition occupancy:
    m  = max over c   : pairwise TT tree, fp32 (exactness of the argmax mask)
    s1 = sum over c   : pairwise TT tree in bf16 (from an ACT bf16 cast)
    s2 = sum over c x^2: pairwise TT tree in bf16 (from ACT Square, bf16)
    per class c: STT (x_c ==) m   -> eq mask + fused count accum
                 STT eq * s1      -> fused S1 accum
                 STT eq * s2      -> fused S2 accum
  Contiguous trees avoid the ~1.6 cyc/elem strided-read penalty of
  tensor_reduce with a strided innermost dim; bf16 tree levels ride the DVE
  2x_1P tensor_tensor mode.  The STT passes are fp32 (the fused
  scalar_tensor_tensor opcode has no fast mode, and bf16 outputs measured
  slower).  Per-(partition, class) partial sums are DMA'd out; host sums the
  tiny partials and applies the final formula.  `target` is unused by the
  reference and never shipped.

  Measured on trn2 via axon: ~252 us HW exec, with the DVE stream fully
  packed (zero >300 ns gaps): ~22 us head (fixed startup + first chunk
  load) + ~226 us DVE + ~10 us tail drain.
"""

import numpy as np


def _ensure_ntff_hook():
    """bass_utils' trace path imports antenv.axon_hooks, which this image
    lacks.  Install a shim backed by trn_agent_boot's ctypes hook so a
    BASS_TRACE=1 environment doesn't crash the run (and tracing works)."""
    import sys
    import types

    try:
        import antenv.axon_hooks  # noqa: F401
        return
    except ImportError:
        pass
    try:
        from trn_agent_boot.trn_boot import _ntff_profile_via_ctypes

        hook = _ntff_profile_via_ctypes("/opt/axon/libaxon_pjrt.so")
    except Exception:
        hook = None
    mod = types.ModuleType("antenv.axon_hooks")
    mod.get_axon_ntff_profile_hook = lambda: hook
    mod.set_axon_ntff_profile_hook = lambda h: None
    sys.modules["antenv.axon_hooks"] = mod

N, C, H, W = 4, 19, 512, 1024
NCORES = 8
SLABS = 4                 # H split into 4 slabs of 128 partitions
P = H // SLABS            # 128
UNITS = [(n, s) for n in range(N) for s in range(SLABS)]   # 16 units
UPC = len(UNITS) // NCORES                                  # 2 units per core
WCHUNK = 512
NCHUNKS = W // WCHUNK

_CACHE = {}

# Per-core chunk schedule: (unit, wchunk-slot, lo, wid).
CHUNKS = [(u, ch, 0, WCHUNK) for u in range(UPC) for ch in range(NCHUNKS)]
SLOT_UNIT = [c[0] for c in CHUNKS]
NSLOTS = len(CHUNKS)


def _build_nc():
    from contextlib import ExitStack

    import concourse.tile as tile
    from concourse import bacc, mybir

    f32 = mybir.dt.float32
    bf16 = mybir.dt.bfloat16
    Alu = mybir.AluOpType
    Act = mybir.ActivationFunctionType

    nc = bacc.Bacc("TRN2", target_bir_lowering=False, debug=False)
    # Host pre-arranges each core's shard as (unit, wchunk, h, c, w) so one
    # chunk load is 128 fully contiguous 38.9 KB runs (descriptor-overhead-
    # bound 2 KB runs cost ~22.8 us/chunk; contiguous is ~13 us).
    x_d = nc.dram_tensor(
        "x", [UPC, NCHUNKS, P, C, WCHUNK], f32, kind="ExternalInput"
    ).ap()
    out_d = nc.dram_tensor(
        "stats", [NSLOTS, P, 3 * C], f32, kind="ExternalOutput"
    ).ap()

    with tile.TileContext(nc) as tc, ExitStack() as ctx:
        xpool = ctx.enter_context(tc.tile_pool(name="x", bufs=2))
        bfpool = ctx.enter_context(tc.tile_pool(name="bf", bufs=1))
        tpool = ctx.enter_context(tc.tile_pool(name="tree", bufs=1))
        eqpool = ctx.enter_context(tc.tile_pool(name="eq", bufs=4))
        jpool = ctx.enter_context(tc.tile_pool(name="junk", bufs=4))
        cpool = ctx.enter_context(tc.tile_pool(name="cols", bufs=2))

        def tree(src, wid, op, dt, out_dt, tag):
            """Pairwise-reduce the C=19 rows of 3-dim AP `src` (P, 19, wid)
            along the row dim via contiguous tensor_tensor ops, no copies:
            leftovers (src row 18, level-1 row 8) are folded in at the end.
            Intermediate levels use dtype dt; the final level writes a
            (P, wid) result of out_dt.  Returns that AP."""
            assert C == 19
            t = tpool.tile([P, 10, wid], dt, tag=tag, name=f"tree_{tag}")
            if dt == out_dt:
                res = t[:, 9, :]
            else:
                res = tpool.tile(
                    [P, wid], out_dt, tag=tag + "o", name=f"tree_{tag}o"
                )[:]
            tt = nc.vector.tensor_tensor
            tt(out=t[:, 0:9, :], in0=src[:, 0:9, :], in1=src[:, 9:18, :], op=op)
            tt(out=t[:, 0:4, :], in0=t[:, 0:4, :], in1=t[:, 4:8, :], op=op)
            tt(out=t[:, 0:2, :], in0=t[:, 0:2, :], in1=t[:, 2:4, :], op=op)
            tt(out=t[:, 0, :], in0=t[:, 0, :], in1=t[:, 1, :], op=op)
            tt(out=t[:, 0, :], in0=t[:, 0, :], in1=t[:, 8, :], op=op)
            tt(out=res, in0=t[:, 0, :], in1=src[:, 18, :], op=op)
            return res

        for slot, (u, ch, lo, wid) in enumerate(CHUNKS):
            xt = xpool.tile([P, C, wid], f32, tag="x", name=f"x{slot}")
            nc.sync.dma_start(xt[:], x_d[u, ch, :, :, lo:lo + wid])

            # bf16 casts on ScalarE (otherwise idle).  Square is issued
            # first and its tree runs before s1's, so at kernel start each
            # tree's input is ready when the m-tree finishes (no DVE stall
            # on the first chunk's ACT latency).
            sq = bfpool.tile([P, C, wid], bf16, tag="sq", name=f"sq{slot}")
            nc.scalar.activation(sq[:], xt[:], Act.Square)
            xb = bfpool.tile([P, C, wid], bf16, tag="xb", name=f"xb{slot}")
            nc.scalar.activation(xb[:], xt[:], Act.Identity)

            m = tree(xt[:], wid, Alu.max, f32, f32, "m")
            s2 = tree(sq[:], wid, Alu.add, bf16, f32, "s2")
            s1 = tree(xb[:], wid, Alu.add, bf16, f32, "s1")

            cols = cpool.tile([P, 3 * C], f32, tag="cols", name=f"cols{slot}")
            for c in range(C):
                eq = eqpool.tile([P, wid], f32, tag="eq", name=f"eq{slot}_{c}")
                nc.vector.scalar_tensor_tensor(
                    out=eq[:], in0=xt[:, c, :], scalar=1.0, in1=m,
                    op0=Alu.mult, op1=Alu.is_equal,
                    accum_out=cols[:, c:c + 1],
                )
                j1 = jpool.tile([P, wid], f32, tag="junk", name=f"j1_{slot}_{c}")
                nc.vector.scalar_tensor_tensor(
                    out=j1[:], in0=eq[:], scalar=1.0, in1=s1,
                    op0=Alu.mult, op1=Alu.mult,
                    accum_out=cols[:, C + c:C + c + 1],
                )
                j2 = jpool.tile([P, wid], f32, tag="junk", name=f"j2_{slot}_{c}")
                nc.vector.scalar_tensor_tensor(
                    out=j2[:], in0=eq[:], scalar=1.0, in1=s2,
                    op0=Alu.mult, op1=Alu.mult,
                    accum_out=cols[:, 2 * C + c:2 * C + c + 1],
                )

            nc.sync.dma_start(out_d[slot], cols[:])

    nc.compile()
    return nc


def _get_nc():
    if "nc" not in _CACHE:
        _CACHE["nc"] = _build_nc()
    return _CACHE["nc"]


def _make_shards(logits):
    shards = []
    for k in range(NCORES):
        units = [UNITS[UPC * k + i] for i in range(UPC)]
        arr = np.stack(
            [logits[n, :, s * P:(s + 1) * P, :] for (n, s) in units]
        ).astype(np.float32, copy=False)            # (UPC, C, P, W)
        arr = arr.reshape(UPC, C, P, NCHUNKS, WCHUNK)
        arr = arr.transpose(0, 3, 2, 1, 4)           # (UPC, NCH, P, C, WC)
        shards.append(np.ascontiguousarray(arr))
    return shards


def _finish(results):
    per_n = np.zeros((N, 3, C), dtype=np.float64)
    for k in range(NCORES):
        arr = np.asarray(results[k]["stats"], dtype=np.float64)
        a = arr.reshape(NSLOTS, P, 3, C).sum(axis=1)   # (NSLOTS, 3, C)
        for slot in range(NSLOTS):
            n, _s = UNITS[UPC * k + SLOT_UNIT[slot]]
            per_n[n] += a[slot]
    cnt, S1, S2 = per_n[:, 0], per_n[:, 1], per_n[:, 2]
    K = np.maximum(cnt, 1.0) * C
    sq_dev = np.maximum(S2 - S1 * S1 / K, 0.0)
    norms = np.where(cnt > 0, np.sqrt(sq_dev), 0.0)
    loss = norms.mean(axis=0).sum()
    return np.array(loss, dtype=np.float32)


def kernel(**inputs):
    _ensure_ntff_hook()
    from concourse.bass_utils import run_bass_kernel_spmd

    logits = np.asarray(inputs["logits"])
    assert logits.shape == (N, C, H, W), logits.shape
    nc = _get_nc()
    shards = _make_shards(logits)
    in_maps = [{"x": shards[k]} for k in range(NCORES)]
    res = run_bass_kernel_spmd(nc, in_maps, list(range(NCORES)))
    return _finish(res.results)



# revision 8
# speedup vs baseline: 1.8199x; 1.8199x over previous
"""CenterLoss (segment_reduce) Trainium2 kernel — TensorE segment-sum version.

Math (faithful to the reference):
  preds = argmax_c logits[n, c, h, w]          (softmax is monotone -> skip it)
  per (n, cls): cnt = #pixels with preds==cls,
                S1 = sum over those pixels of sum_c x,
                S2 = sum over those pixels of sum_c x^2
  K = max(cnt,1)*C; sq_dev = max(S2 - S1^2/K, 0)
  loss = sum_cls mean_n( cnt>0 ? sqrt(sq_dev) : 0 )

Device strategy (8 cores, data-parallel over 16 units = (n, H-slab of 128)):
  Host pre-casts logits to bf16 (halves HBM traffic; bf16-argmax ties touch
  ~0.7% of pixels and shift the loss by ~4e-3 rel — within the 2e-2 gate).
  Each core takes 2 units as (128h partitions, [xb(19) | x^2(19) | 1] x 512w)
  "xs" tiles:
    ScalarE:  Square(xb) -> xs[:, 19:38, :]
    GpSimd:   ones row memset
    DVE:      m = max over c (pairwise bf16 tree, 2x mode)
              E = (xb == m) one-hot, ONE broadcast tensor_tensor (2x mode)
    TensorE:  per 4 w-columns, matmul(psum += E_grpT(128x76) @ xs_grp(128x156))
              PSUM-accumulated over the whole unit.  Off-diagonal (w,w')
              blocks are junk and ignored; diagonal blocks hold, per class,
              the per-channel sums of xb and x^2 plus the pixel count.
  The per-unit (76,156) PSUM is copied to SBUF (ScalarE) and DMA'd out;
  host extracts diagonal blocks and applies the final sqrt/mean formula.

  vs. the previous all-DVE version (57 scalar_tensor_tensor passes/chunk,
  ~250us): DVE now runs ~11us/chunk and the segment reduction rides the
  otherwise-idle TensorE.
"""

import numpy as np
import ml_dtypes


def _ensure_ntff_hook():
    """bass_utils' trace path imports antenv.axon_hooks, which this image
    lacks.  Install a shim backed by trn_agent_boot's ctypes hook so a
    BASS_TRACE=1 environment doesn't crash the run (and tracing works)."""
    import sys
    import types

    try:
        import antenv.axon_hooks  # noqa: F401
        return
    except ImportError:
        pass
    try:
        from trn_agent_boot.trn_boot import _ntff_profile_via_ctypes

        hook = _ntff_profile_via_ctypes("/opt/axon/libaxon_pjrt.so")
    except Exception:
        hook = None
    mod = types.ModuleType("antenv.axon_hooks")
    mod.get_axon_ntff_profile_hook = lambda: hook
    mod.set_axon_ntff_profile_hook = lambda h: None
    sys.modules["antenv.axon_hooks"] = mod

N, C, H, W = 4, 19, 512, 1024
NCORES = 8
SLABS = 4                 # H split into 4 slabs of 128 partitions
P = H // SLABS            # 128
UNITS = [(n, s) for n in range(N) for s in range(SLABS)]   # 16 units
UPC = len(UNITS) // NCORES                                  # 2 units per core
WCHUNK = 512
NCHUNKS = W // WCHUNK
WG = 3                    # concurrent col-tiled matmuls (PSUM slices at 0/32/64)
NCOLS = 2 * C + 1         # 39 psum columns: [xb-chans | sq-chans | count]

_CACHE = {}


def _build_nc():
    from contextlib import ExitStack

    import concourse.tile as tile
    from concourse import bacc, mybir

    f32 = mybir.dt.float32
    bf16 = mybir.dt.bfloat16
    Alu = mybir.AluOpType
    Act = mybir.ActivationFunctionType

    nc = bacc.Bacc("TRN2", target_bir_lowering=False, debug=False)
    # Host pre-arranges each core's shard as (unit, wchunk, h, c, w) bf16 so
    # one chunk load is 128 fully contiguous 19.5 KB runs.
    x_d = nc.dram_tensor(
        "x", [UPC, NCHUNKS, P, C, WCHUNK], bf16, kind="ExternalInput"
    ).ap()
    out_d = nc.dram_tensor(
        "stats", [UPC, 128, NCOLS], f32, kind="ExternalOutput"
    ).ap()

    with tile.TileContext(nc) as tc, ExitStack() as ctx:
        xpool = ctx.enter_context(tc.tile_pool(name="xs", bufs=2))
        epool = ctx.enter_context(tc.tile_pool(name="eq", bufs=2))
        tpool = ctx.enter_context(tc.tile_pool(name="tree", bufs=2))
        ppool = ctx.enter_context(tc.tile_pool(name="psum", bufs=2, space="PSUM"))
        spool = ctx.enter_context(tc.tile_pool(name="sb_out", bufs=2))

        def tree(src, tag):
            """Pairwise max-reduce the C=19 rows of 3-dim AP `src`
            (P, 19, WCHUNK) along the row dim via contiguous bf16
            tensor_tensor ops (2x DVE mode); leftovers (src row 18,
            level-1 row 8) fold in at the end.  Returns a (P, WCHUNK) AP."""
            assert C == 19
            t = tpool.tile([P, 10, WCHUNK], bf16, tag="tree", name=f"tree_{tag}")
            tt = nc.vector.tensor_tensor
            op = Alu.max
            tt(out=t[:, 0:9, :], in0=src[:, 0:9, :], in1=src[:, 9:18, :], op=op)
            tt(out=t[:, 0:4, :], in0=t[:, 0:4, :], in1=t[:, 4:8, :], op=op)
            tt(out=t[:, 0:2, :], in0=t[:, 0:2, :], in1=t[:, 2:4, :], op=op)
            tt(out=t[:, 0, :], in0=t[:, 0, :], in1=t[:, 1, :], op=op)
            tt(out=t[:, 0, :], in0=t[:, 0, :], in1=t[:, 8, :], op=op)
            tt(out=t[:, 9, :], in0=t[:, 0, :], in1=src[:, 18, :], op=op)
            return t[:, 9, :]

        psum_t = [None] * UPC
        for u in range(UPC):
            for ch in range(NCHUNKS):
                slot = u * NCHUNKS + ch
                xs = xpool.tile([P, 2 * C + 1, WCHUNK], bf16, tag="xs",
                                name=f"xs{slot}")
                nc.sync.dma_start(xs[:, 0:C, :], x_d[u, ch])
                nc.gpsimd.memset(xs[:, 2 * C, :], 1.0)
                nc.scalar.activation(xs[:, C:2 * C, :], xs[:, 0:C, :],
                                     Act.Square)

                m = tree(xs[:, 0:C, :], f"m{slot}")
                eq = epool.tile([P, C, WCHUNK], bf16, tag="eq", name=f"eq{slot}")
                m_b = m.unsqueeze(1).broadcast_to([P, C, WCHUNK])
                nc.vector.tensor_tensor(out=eq[:], in0=xs[:, 0:C, :], in1=m_b,
                                        op=Alu.is_equal)

                if ch == 0:
                    psum_t[u] = ppool.tile([128, NCOLS], f32, tag="ps",
                                           name=f"ps{u}")
                for w in range(WCHUNK):
                    j = w % WG
                    nc.tensor.matmul(
                        psum_t[u][32 * j:32 * j + C, :],
                        eq[:, :, w], xs[:, :, w],
                        start=(ch == 0 and w < WG),
                        stop=(ch == NCHUNKS - 1 and w == WCHUNK - 1),
                        skip_group_check=True,
                    )

            sb = spool.tile([128, NCOLS], f32, tag="sb", name=f"sb{u}")
            nc.scalar.copy(out=sb[:], in_=psum_t[u][:, :])
            nc.sync.dma_start(out_d[u], sb[:])

    nc.compile()
    return nc


def _get_nc():
    if "nc" not in _CACHE:
        _CACHE["nc"] = _build_nc()
    return _CACHE["nc"]


def _make_shards(logits):
    xb = np.asarray(logits).astype(ml_dtypes.bfloat16, copy=False)
    shards = []
    for k in range(NCORES):
        units = [UNITS[UPC * k + i] for i in range(UPC)]
        arr = np.stack(
            [xb[n, :, s * P:(s + 1) * P, :] for (n, s) in units]
        )                                            # (UPC, C, P, W)
        arr = arr.reshape(UPC, C, P, NCHUNKS, WCHUNK)
        arr = arr.transpose(0, 3, 2, 1, 4)           # (UPC, NCH, P, C, WC)
        shards.append(np.ascontiguousarray(arr))
    return shards


def _finish(results):
    S1 = np.zeros((N, C), dtype=np.float64)
    S2 = np.zeros((N, C), dtype=np.float64)
    cnt = np.zeros((N, C), dtype=np.float64)
    for k in range(NCORES):
        arr = np.asarray(results[k]["stats"], dtype=np.float64)
        for u in range(UPC):
            n, _s = UNITS[UPC * k + u]
            for j in range(WG):
                blk = arr[u, 32 * j:32 * j + C, :]
                S1[n] += blk[:, 0:C].sum(axis=1)
                S2[n] += blk[:, C:2 * C].sum(axis=1)
                cnt[n] += blk[:, 2 * C]
    K = np.maximum(cnt, 1.0) * C
    sq_dev = np.maximum(S2 - S1 * S1 / K, 0.0)
    norms = np.where(cnt > 0, np.sqrt(sq_dev), 0.0)
    loss = norms.mean(axis=0).sum()
    return np.array(loss, dtype=np.float32)


def kernel(**inputs):
    _ensure_ntff_hook()
    from concourse.bass_utils import run_bass_kernel_spmd

    logits = np.asarray(inputs["logits"])
    assert logits.shape == (N, C, H, W), logits.shape
    nc = _get_nc()
    shards = _make_shards(logits)
    in_maps = [{"x": shards[k]} for k in range(NCORES)]
    res = run_bass_kernel_spmd(nc, in_maps, list(range(NCORES)))
    return _finish(res.results)
